# revision 22
# baseline (speedup 1.0000x reference)
"""Trainium2 Bass kernel: FlowNet-style local correlation (9x9 window) + softmax.

Computes, for inputs x,y [B=4, C=1024, H=96, W=96]:
  q = conv1x1(y; query_w, query_b)   # [B, 256, H, W]
  k = conv1x1(x; key_w,  key_b)      # [B, 256, H, W]
  corr[b,di,dj,h,w] = sum_c q[b,c,h,w] * kpad[b,c,h+di,w+dj] / 256
  out = softmax(corr over the 81 (di,dj) channels)  # [B, H, W, 81]

Sharding: 8 cores = 4 batches x 2 H-halves (48 rows each, 4-row halo on the
k side, handled by host-side zero padding + row-masked key bias).

Per-core kernel (W-COLUMN correlation scheme):
  - SWDGE cast-DMA loads (fp32 HBM -> bf16 SBUF), 8-row stages
  - projections on TensorE: K=1024 contraction in 8 PSUM-accumulated chunks;
    1/C4 normalization folded into the query weights+bias on host
  - correlation per output COLUMN w and H-half: lhsT = q[:, h0:h0+24, w]
    (stationary, M=24), rhs = k rows [h0, h0+32) x 9 shifted cols
    -> [128, 288] streamed in one matmul per C4-chunk, PSUM-accumulated
  - band extraction: for pixel (h,w) the 81 needed scores are the
    CONTIGUOUS columns [9h, 9h+81) of score row h (shear slope 9).
    scores -> DRAM contiguous [24, 288]; strided re-read with row pitch
    297 yields the [24, 81] band directly (81-element contiguous runs).
  - softmax on 4 packed half-columns [96, 81]: ScalarE exp with fused
    row-sum, VectorE reciprocal and scale. (Max-subtraction skipped:
    logits are O(1).)
  - H-split x2 pipelining: correlation of half A overlaps the input
    DMA + projection of half B.
"""

import numpy as np

import concourse.bacc as bacc
import concourse.bass as bass
import concourse.mybir as mybir
import concourse.tile as tile
from concourse.bass_utils import run_bass_kernel_spmd

F32 = mybir.dt.float32
BF16 = mybir.dt.bfloat16
AF = mybir.ActivationFunctionType

B, C, H, W = 4, 1024, 96, 96
C4 = 256
D = 4                # max displacement
ND = 2 * D + 1       # 9
NB = ND * ND         # 81
HH = H // 2          # 48 rows per core
KR = HH + 2 * D      # 56 k rows incl. halo/pad
WP = W + 2 * D       # 104 padded k width
CC = C // 128        # 8 contraction chunks
MC = C4 // 128       # 2 output-channel chunks
RG = 8               # rows per input stage
N_CORES = 8

NSPLIT = 2           # H-halves for pipelining
NH = HH // NSPLIT    # 24 output rows per half
NKR = NH + 2 * D     # 32 k rows per half
NS = NKR * ND        # 288 score columns per (column, half)
SHR = NS + ND        # 297 sheared read pitch (read span 23*297+81 = 6912)
CPACK = 4            # columns packed per PE col-group batch (4*24 = 96 parts)
CB = 8 * NS + NH * SHR   # 9432: per-column scratch block pitch — fits the
                         # 32-row (incl. 8 pad rows) write and the sheared
                         # [NH, SHR] re-read at row offset up to 8


def _build_tile(tc, xs, ys, wqt, wkt, bqr, bkr, out):
    nc = tc.nc
    with (
        tc.tile_pool(name="const", bufs=1) as const,
        tc.tile_pool(name="big", bufs=1) as big,
        tc.tile_pool(name="stage", bufs=3) as stage,
        tc.tile_pool(name="spool", bufs=4) as spool,
        tc.tile_pool(name="opool", bufs=3) as opool,
        tc.tile_pool(name="psq", bufs=4, space="PSUM") as psq,
        tc.tile_pool(name="psA", bufs=1, space="PSUM") as psA,
        tc.tile_pool(name="dram", bufs=4, space="DRAM") as dram,
    ):
        # --- constants: weights (transposed + q pre-scaled on host), biases ---
        wq_sb = const.tile([128, CC, C4], BF16)
        nc.gpsimd.dma_start(wq_sb[:], wqt.rearrange("(cc p) o -> p cc o", p=128))
        wk_sb = const.tile([128, CC, C4], BF16)
        nc.gpsimd.dma_start(wk_sb[:], wkt.rearrange("(cc p) o -> p cc o", p=128))
        bqr_sb = const.tile([128, MC, HH], F32)
        nc.gpsimd.dma_start(bqr_sb[:], bqr.rearrange("(m p) r -> p m r", p=128))
        bkr_sb = const.tile([128, MC, KR], F32)
        nc.gpsimd.dma_start(bkr_sb[:], bkr.rearrange("(m p) r -> p m r", p=128))

        q_sb = big.tile([128, MC, HH, W], BF16)
        k_sb = big.tile([128, MC, KR, WP], BF16)
        # zero fill: provides the 4-col zero pads (halo rows are projected
        # from host-zero-padded x with masked bias, so they come out zero)
        nc.vector.memset(k_sb[:], 0.0)

        def project(src, w_sb, br_sb, row0, nrows, is_q):
            # project `nrows` rows starting at row0 (src row offset == row0)
            for j in range(nrows // RG):
                st = stage.tile([128, CC, RG * W], BF16, tag="stage")
                nc.gpsimd.dma_start(
                    st[:],
                    src[:, row0 + j * RG:row0 + (j + 1) * RG, :].rearrange(
                        "(cc p) h w -> p cc (h w)", p=128
                    ),
                )
                for g in range(2):
                    for m in range(MC):
                        # full-bank (512 f32) tiles keep every PSUM slot
                        # bank-aligned so accumulation groups never share
                        # a bank's has_written zero region
                        ps = psq.tile([128, 512], F32)
                        for cc in range(CC):
                            nc.tensor.matmul(
                                ps[:, 0:RG * W // 2],
                                w_sb[:, cc, m * 128:(m + 1) * 128],
                                st[:, cc, g * 384:(g + 1) * 384],
                                start=(cc == 0),
                                stop=(cc == CC - 1),
                            )
                        r0 = row0 + j * RG + g * 4
                        bias3 = (
                            br_sb[:, m, r0:r0 + 4]
                            .unsqueeze(-1)
                            .broadcast_to((128, 4, W))
                        )
                        if is_q:
                            dst = q_sb[:, m, r0:r0 + 4, :]
                        else:
                            dst = k_sb[:, m, r0:r0 + 4, D:D + W]
                        nc.vector.tensor_tensor(
                            dst,
                            ps[:, 0:RG * W // 2].rearrange(
                                "p (a b) -> p a b", b=W
                            ),
                            bias3,
                            op=mybir.AluOpType.add,
                        )

        def correlate(half):
            h0 = half * NH      # k rows [h0, h0 + NKR); output rows h0..h0+NH
            qr0 = h0 - (8 if half else 0)   # 32 projected q rows per group;
            ro = h0 - qr0                   # real rows sit at offset ro..ro+24
            for wg in range(W // CPACK):
                sd = dram.tile([CPACK * CB], BF16, tag="sd")
                # 4 columns ride concurrent PE column-groups (32*ci..32*ci+32),
                # each accumulating in its own PSUM bank
                s_sb = spool.tile([128, NS], BF16, tag="s")
                for ci in range(CPACK):
                    w = wg * CPACK + ci
                    ps = psA.tile([128, 512], F32, tag=f"psA{ci}")
                    for m in range(MC):
                        nc.tensor.matmul(
                            ps[32 * ci:32 * (ci + 1), 0:NS],
                            q_sb[:, m, qr0:qr0 + 32, w],
                            k_sb[:, m, h0:h0 + NKR, w:w + ND],
                            start=(m == 0),
                            stop=(m == MC - 1),
                            tile_position=(0, 32 * ci),
                        )
                    nc.vector.tensor_copy(
                        s_sb[32 * ci:32 * (ci + 1), :],
                        ps[32 * ci:32 * (ci + 1), 0:NS],
                    )
                nc.sync.dma_start(
                    sd[:].rearrange("(c y) -> c y", y=CB)[:, 0:32 * NS]
                    .rearrange("c (r x) -> c r x", x=NS),
                    s_sb[:],
                )
                # shear re-read: band[hl, dd] = s[ro + hl, 9*hl + dd]
                band = spool.tile([CPACK * NH, NB], BF16, tag="band")
                gsrc = (
                    sd[:].rearrange("(c y) -> c y", y=CB)
                    [:, ro * NS:ro * NS + NH * SHR]
                    .rearrange("c (h x) -> c h x", x=SHR)[:, :, 0:NB]
                )
                nc.sync.dma_start(band[:], gsrc)

                p_sb = opool.tile([CPACK * NH, NB], F32, tag="p")
                ssum = opool.tile([CPACK * NH, 1], F32, tag="ssum")
                nc.scalar.activation(p_sb[:], band[:], AF.Exp, accum_out=ssum[:])
                rinv = opool.tile([CPACK * NH, 1], F32, tag="rinv")
                nc.vector.reciprocal(rinv[:], ssum[:])
                o_sb = opool.tile([CPACK * NH, NB], F32, tag="o")
                nc.vector.tensor_scalar_mul(o_sb[:], p_sb[:], rinv[:])
                nc.sync.dma_start(
                    out[h0:h0 + NH, wg * CPACK:(wg + 1) * CPACK, :].rearrange(
                        "h c d -> c h d"
                    ),
                    o_sb[:],
                )

        # --- pipelined halves: proj(A), [corr(A) || proj(B)], corr(B) ---
        # corr(0) reads q rows [0, 32) (8 extra pad rows), corr(1) [16, 48)
        project(xs, wk_sb, bkr_sb, 0, NKR, is_q=False)
        project(ys, wq_sb, bqr_sb, 0, 32, is_q=True)
        correlate(0)
        project(xs, wk_sb, bkr_sb, NKR, KR - NKR, is_q=False)
        project(ys, wq_sb, bqr_sb, 32, HH - 32, is_q=True)
        correlate(1)


def build_bass():
    nc = bacc.Bacc("TRN2", target_bir_lowering=False, debug=False,
                   num_devices=N_CORES)
    xs = nc.dram_tensor("xs", [C, KR, W], F32, kind="ExternalInput")
    ys = nc.dram_tensor("ys", [C, HH, W], F32, kind="ExternalInput")
    wqt = nc.dram_tensor("wqt", [C, C4], F32, kind="ExternalInput")
    wkt = nc.dram_tensor("wkt", [C, C4], F32, kind="ExternalInput")
    bqr = nc.dram_tensor("bqr", [C4, HH], F32, kind="ExternalInput")
    bkr = nc.dram_tensor("bkr", [C4, KR], F32, kind="ExternalInput")
    out = nc.dram_tensor("out", [HH, W, NB], F32, kind="ExternalOutput")
    with tile.TileContext(nc) as tc:
        _build_tile(tc, xs.ap(), ys.ap(), wqt.ap(), wkt.ap(), bqr.ap(),
                    bkr.ap(), out.ap())
    nc.compile()
    return nc


def make_in_maps(x, y, query_w, query_b, key_w, key_b):
    x = np.asarray(x, dtype=np.float32)
    y = np.asarray(y, dtype=np.float32)
    xp = np.pad(x, ((0, 0), (0, 0), (D, D), (0, 0)))
    # fold the 1/C4 correlation normalization into the query projection
    wqt = (np.asarray(query_w, np.float32).T / C4).astype(np.float32)
    wqt = np.ascontiguousarray(wqt)
    wkt = np.ascontiguousarray(np.asarray(key_w, np.float32).T)
    bqr = np.ascontiguousarray(np.repeat(
        (np.asarray(query_b, np.float32) / C4)[:, None], HH, axis=1))
    kb = np.asarray(key_b, np.float32)
    in_maps = []
    for core in range(N_CORES):
        b, half = divmod(core, 2)
        h0 = half * HH
        rows = np.arange(KR) + h0 - D
        mask = ((rows >= 0) & (rows < H)).astype(np.float32)
        in_maps.append({
            "xs": np.ascontiguousarray(xp[b, :, h0:h0 + KR, :]),
            "ys": np.ascontiguousarray(y[b, :, h0:h0 + HH, :]),
            "wqt": wqt,
            "wkt": wkt,
            "bqr": bqr,
            "bkr": np.ascontiguousarray(kb[:, None] * mask[None, :]),
        })
    return in_maps


_NC = None


def _get_nc():
    global _NC
    if _NC is None:
        _NC = build_bass()
    return _NC


def kernel(x, y, query_w, query_b, key_w, key_b, _trace=False):
    nc = _get_nc()
    in_maps = make_in_maps(x, y, query_w, query_b, key_w, key_b)
    res = run_bass_kernel_spmd(nc, in_maps, core_ids=list(range(N_CORES)),
                               trace=_trace)
    out = np.empty((B, H, W, NB), np.float32)
    for core in range(N_CORES):
        b, half = divmod(core, 2)
        out[b, half * HH:(half + 1) * HH] = res.results[core]["out"]
    if _trace:
        kernel.last_results = res
    return out


# revision 25
# speedup vs baseline: 296.6913x; 296.6913x over previous
"""Trainium2 Bass kernel: FlowNet-style local correlation (9x9 window) + softmax.

Computes, for inputs x,y [B=4, C=1024, H=96, W=96]:
  q = conv1x1(y; query_w, query_b)   # [B, 256, H, W]
  k = conv1x1(x; key_w,  key_b)      # [B, 256, H, W]
  corr[b,di,dj,h,w] = sum_c q[b,c,h,w] * kpad[b,c,h+di,w+dj] / 256
  out = softmax(corr over the 81 (di,dj) channels)  # [B, H, W, 81]

Sharding: 8 cores = 4 batches x 2 H-halves (48 rows each, 4-row halo on the
k side, handled by host-side zero padding + row-masked key bias).

Per-core kernel (W-COLUMN correlation scheme):
  - SWDGE cast-DMA loads (fp32 HBM -> bf16 SBUF), 8-row stages
  - projections on TensorE: K=1024 contraction in 8 PSUM-accumulated chunks;
    1/C4 normalization folded into the query weights+bias on host
  - correlation per output COLUMN w and H-half: lhsT = q[:, h0:h0+24, w]
    (stationary, M=24), rhs = k rows [h0, h0+32) x 9 shifted cols
    -> [128, 288] streamed in one matmul per C4-chunk, PSUM-accumulated
  - band extraction: for pixel (h,w) the 81 needed scores are the
    CONTIGUOUS columns [9h, 9h+81) of score row h (shear slope 9).
    scores -> DRAM contiguous [24, 288]; strided re-read with row pitch
    297 yields the [24, 81] band directly (81-element contiguous runs).
  - softmax on 4 packed half-columns [96, 81]: ScalarE exp with fused
    row-sum, VectorE reciprocal and scale. (Max-subtraction skipped:
    logits are O(1).)
  - H-split x2 pipelining: correlation of half A overlaps the input
    DMA + projection of half B.
"""

import numpy as np

import concourse.bacc as bacc
import concourse.bass as bass
import concourse.mybir as mybir
import concourse.tile as tile
from concourse.bass_utils import run_bass_kernel_spmd

F32 = mybir.dt.float32
BF16 = mybir.dt.bfloat16
AF = mybir.ActivationFunctionType

B, C, H, W = 4, 1024, 96, 96
C4 = 256
D = 4                # max displacement
ND = 2 * D + 1       # 9
NB = ND * ND         # 81
HH = H // 2          # 48 rows per core
KR = HH + 2 * D      # 56 k rows incl. halo/pad
WP = W + 2 * D       # 104 padded k width
CC = C // 128        # 8 contraction chunks
MC = C4 // 128       # 2 output-channel chunks
RG = 8               # rows per input stage
N_CORES = 8

NSPLIT = 2           # H-halves for pipelining
NH = HH // NSPLIT    # 24 output rows per half
NKR = NH + 2 * D     # 32 k rows per half
NS = NKR * ND        # 288 score columns per (column, half)
SHR = NS + ND        # 297 sheared read pitch (read span 23*297+81 = 6912)
CPACK = 4            # columns packed per PE col-group batch (4*24 = 96 parts)
CB = 8 * NS + NH * SHR   # 9432: per-column scratch block pitch — fits the
                         # 32-row (incl. 8 pad rows) write and the sheared
                         # [NH, SHR] re-read at row offset up to 8


def _build_tile(tc, xs, ys, wqt, wkt, bqr, bkr, out, loop=1):
    nc = tc.nc
    from contextlib import nullcontext
    with (
        tc.tile_pool(name="const", bufs=1) as const,
        tc.tile_pool(name="big", bufs=1) as big,
        tc.tile_pool(name="stage", bufs=3) as stage,
        tc.tile_pool(name="spool", bufs=4) as spool,
        tc.tile_pool(name="opool", bufs=3) as opool,
        tc.tile_pool(name="psq", bufs=4, space="PSUM") as psq,
        tc.tile_pool(name="psA", bufs=1, space="PSUM") as psA,
        tc.tile_pool(name="dram", bufs=4, space="DRAM") as dram,
        tc.For_i(0, loop, 1) if loop > 1 else nullcontext(),
    ):
        # --- constants: weights (transposed + q pre-scaled on host), biases ---
        wq_sb = const.tile([128, CC, C4], BF16)
        nc.gpsimd.dma_start(wq_sb[:], wqt.rearrange("(cc p) o -> p cc o", p=128))
        wk_sb = const.tile([128, CC, C4], BF16)
        nc.gpsimd.dma_start(wk_sb[:], wkt.rearrange("(cc p) o -> p cc o", p=128))
        bqr_sb = const.tile([128, MC, HH], F32)
        nc.gpsimd.dma_start(bqr_sb[:], bqr.rearrange("(m p) r -> p m r", p=128))
        bkr_sb = const.tile([128, MC, KR], F32)
        nc.gpsimd.dma_start(bkr_sb[:], bkr.rearrange("(m p) r -> p m r", p=128))

        q_sb = big.tile([128, MC, HH, W], BF16)
        k_sb = big.tile([128, MC, KR, WP], BF16)
        # zero fill: provides the 4-col zero pads (halo rows are projected
        # from host-zero-padded x with masked bias, so they come out zero)
        nc.vector.memset(k_sb[:], 0.0)

        def project(src, w_sb, br_sb, row0, nrows, is_q):
            # project `nrows` rows starting at row0 (src row offset == row0)
            for j in range(nrows // RG):
                st = stage.tile([128, CC, RG * W], BF16, tag="stage")
                nc.gpsimd.dma_start(
                    st[:],
                    src[:, row0 + j * RG:row0 + (j + 1) * RG, :].rearrange(
                        "(cc p) h w -> p cc (h w)", p=128
                    ),
                )
                for g in range(2):
                    for m in range(MC):
                        # full-bank (512 f32) tiles keep every PSUM slot
                        # bank-aligned so accumulation groups never share
                        # a bank's has_written zero region
                        ps = psq.tile([128, 512], F32)
                        for cc in range(CC):
                            nc.tensor.matmul(
                                ps[:, 0:RG * W // 2],
                                w_sb[:, cc, m * 128:(m + 1) * 128],
                                st[:, cc, g * 384:(g + 1) * 384],
                                start=(cc == 0),
                                stop=(cc == CC - 1),
                            )
                        r0 = row0 + j * RG + g * 4
                        bias3 = (
                            br_sb[:, m, r0:r0 + 4]
                            .unsqueeze(-1)
                            .broadcast_to((128, 4, W))
                        )
                        if is_q:
                            dst = q_sb[:, m, r0:r0 + 4, :]
                        else:
                            dst = k_sb[:, m, r0:r0 + 4, D:D + W]
                        nc.vector.tensor_tensor(
                            dst,
                            ps[:, 0:RG * W // 2].rearrange(
                                "p (a b) -> p a b", b=W
                            ),
                            bias3,
                            op=mybir.AluOpType.add,
                        )

        def correlate(half):
            h0 = half * NH      # k rows [h0, h0 + NKR); output rows h0..h0+NH
            qr0 = h0 - (8 if half else 0)   # 32 projected q rows per group;
            ro = h0 - qr0                   # real rows sit at offset ro..ro+24
            for wg in range(W // CPACK):
                sd = dram.tile([CPACK * CB], BF16, tag="sd")
                # 4 columns ride concurrent PE column-groups (32*ci..32*ci+32),
                # each accumulating in its own PSUM bank
                s_sb = spool.tile([128, NS], BF16, tag="s")
                for ci in range(CPACK):
                    w = wg * CPACK + ci
                    ps = psA.tile([128, 512], F32, tag=f"psA{ci}")
                    for m in range(MC):
                        nc.tensor.matmul(
                            ps[32 * ci:32 * (ci + 1), 0:NS],
                            q_sb[:, m, qr0:qr0 + 32, w],
                            k_sb[:, m, h0:h0 + NKR, w:w + ND],
                            start=(m == 0),
                            stop=(m == MC - 1),
                            tile_position=(0, 32 * ci),
                        )
                    nc.vector.tensor_copy(
                        s_sb[32 * ci:32 * (ci + 1), :],
                        ps[32 * ci:32 * (ci + 1), 0:NS],
                    )
                nc.sync.dma_start(
                    sd[:].rearrange("(c y) -> c y", y=CB)[:, 0:32 * NS]
                    .rearrange("c (r x) -> c r x", x=NS),
                    s_sb[:],
                )
                # shear re-read: band[hl, dd] = s[ro + hl, 9*hl + dd]
                band = spool.tile([CPACK * NH, NB], BF16, tag="band")
                gsrc = (
                    sd[:].rearrange("(c y) -> c y", y=CB)
                    [:, ro * NS:ro * NS + NH * SHR]
                    .rearrange("c (h x) -> c h x", x=SHR)[:, :, 0:NB]
                )
                nc.sync.dma_start(band[:], gsrc)

                p_sb = opool.tile([CPACK * NH, NB], F32, tag="p")
                ssum = opool.tile([CPACK * NH, 1], F32, tag="ssum")
                nc.scalar.activation(p_sb[:], band[:], AF.Exp, accum_out=ssum[:])
                rinv = opool.tile([CPACK * NH, 1], F32, tag="rinv")
                nc.vector.reciprocal(rinv[:], ssum[:])
                o_sb = opool.tile([CPACK * NH, NB], F32, tag="o")
                nc.vector.tensor_scalar_mul(o_sb[:], p_sb[:], rinv[:])
                nc.sync.dma_start(
                    out[h0:h0 + NH, wg * CPACK:(wg + 1) * CPACK, :].rearrange(
                        "h c d -> c h d"
                    ),
                    o_sb[:],
                )

        # --- pipelined halves: proj(A), [corr(A) || proj(B)], corr(B) ---
        # corr(0) reads q rows [0, 32) (8 extra pad rows), corr(1) [16, 48)
        project(xs, wk_sb, bkr_sb, 0, NKR, is_q=False)
        project(ys, wq_sb, bqr_sb, 0, 32, is_q=True)
        correlate(0)
        project(xs, wk_sb, bkr_sb, NKR, KR - NKR, is_q=False)
        project(ys, wq_sb, bqr_sb, 32, HH - 32, is_q=True)
        correlate(1)


def build_bass(loop=1):
    nc = bacc.Bacc("TRN2", target_bir_lowering=False, debug=False,
                   num_devices=N_CORES)
    xs = nc.dram_tensor("xs", [C, KR, W], F32, kind="ExternalInput")
    ys = nc.dram_tensor("ys", [C, HH, W], F32, kind="ExternalInput")
    wqt = nc.dram_tensor("wqt", [C, C4], F32, kind="ExternalInput")
    wkt = nc.dram_tensor("wkt", [C, C4], F32, kind="ExternalInput")
    bqr = nc.dram_tensor("bqr", [C4, HH], F32, kind="ExternalInput")
    bkr = nc.dram_tensor("bkr", [C4, KR], F32, kind="ExternalInput")
    out = nc.dram_tensor("out", [HH, W, NB], F32, kind="ExternalOutput")
    with tile.TileContext(nc) as tc:
        _build_tile(tc, xs.ap(), ys.ap(), wqt.ap(), wkt.ap(), bqr.ap(),
                    bkr.ap(), out.ap(), loop=loop)
    nc.compile()
    return nc


def make_in_maps(x, y, query_w, query_b, key_w, key_b):
    x = np.asarray(x, dtype=np.float32)
    y = np.asarray(y, dtype=np.float32)
    xp = np.pad(x, ((0, 0), (0, 0), (D, D), (0, 0)))
    # fold the 1/C4 correlation normalization into the query projection
    wqt = (np.asarray(query_w, np.float32).T / C4).astype(np.float32)
    wqt = np.ascontiguousarray(wqt)
    wkt = np.ascontiguousarray(np.asarray(key_w, np.float32).T)
    bqr = np.ascontiguousarray(np.repeat(
        (np.asarray(query_b, np.float32) / C4)[:, None], HH, axis=1))
    kb = np.asarray(key_b, np.float32)
    in_maps = []
    for core in range(N_CORES):
        b, half = divmod(core, 2)
        h0 = half * HH
        rows = np.arange(KR) + h0 - D
        mask = ((rows >= 0) & (rows < H)).astype(np.float32)
        in_maps.append({
            "xs": np.ascontiguousarray(xp[b, :, h0:h0 + KR, :]),
            "ys": np.ascontiguousarray(y[b, :, h0:h0 + HH, :]),
            "wqt": wqt,
            "wkt": wkt,
            "bqr": bqr,
            "bkr": np.ascontiguousarray(kb[:, None] * mask[None, :]),
        })
    return in_maps


_NC = None


def _get_nc():
    global _NC
    if _NC is None:
        _NC = build_bass()
    return _NC


def kernel(x, y, query_w, query_b, key_w, key_b, _trace=False):
    nc = _get_nc()
    in_maps = make_in_maps(x, y, query_w, query_b, key_w, key_b)
    res = run_bass_kernel_spmd(nc, in_maps, core_ids=list(range(N_CORES)),
                               trace=_trace)
    out = np.empty((B, H, W, NB), np.float32)
    for core in range(N_CORES):
        b, half = divmod(core, 2)
        out[b, half * HH:(half + 1) * HH] = res.results[core]["out"]
    if _trace:
        kernel.last_results = res
    return out
